# revision 34
# baseline (speedup 1.0000x reference)
"""Causal self-attention kernel for TRN2 (8 NeuronCores, Bass/Tile).

Problem: B=8, T=1024, C=768, H=12, HD=64.
  qkv = x @ W_attn + b_attn ; causal softmax attention ; y = att_out @ W_proj + b_proj

Sharding: pure data-parallel over batch — core b computes batch element b.

Per-core dataflow (all matmuls bf16):
  xT   [768,1024]  = PE-transpose of x                      (lhsT/rhs source)
  qkT  [1536,1024] = (W_qk)^T-style projection: qkT[c',t] = sum_c W[c,c'] xT[c,t]
                     (+ b_qk folded in as per-partition scalar in the drain)
  V    [1024,768]  : V[t,c'] = sum_c xT[c,t] W_v[c,c']      (per-head Vp tiles with
                     a leading ones column -> PV matmul also produces Z row)
  per head h, i-block (512 cols):
     ST[j,i] = kT^T q  (K=64, causal-trimmed)   -> exp(0.125*ST) on ScalarE
     tri-mask on diagonal 128x128 sub-block (multiplicative, post-exp)
     OT'[0:64,:] = unnormalized attention out (transposed), OT'[64,:] = Z
     ATn[c,t] = OT'[0:64]/Z via fused scalar_tensor_tensor drain from PSUM
  y[t,:] = ATn^T-contraction with W_proj + bp_eff
  bias handling: b_qk in qkT drain; b_v and b_proj folded on the HOST into
  bp_eff = b_v @ W_proj + b_proj (v-bias composes linearly through proj since
  softmax rows sum to 1), added in the y drain from a broadcast tile.
"""

import numpy as np

import concourse.bass as bass
import concourse.mybir as mybir
import concourse.tile as tile
from concourse import bacc
from concourse.bass_utils import run_bass_kernel_spmd

F32 = mybir.dt.float32
BF16 = mybir.dt.bfloat16
AF = mybir.ActivationFunctionType
ALU = mybir.AluOpType

T, C, H, HD = 1024, 768, 12, 64
NCORES = 8
CC = C // 128          # 6 contraction chunks
TP = T // 128          # 8 t-chunks of 128
TB = T // 512          # 2 t-blocks of 512
QKCP = 2 * C // 128    # 12 qkT partition tiles
SCALE = 1.0 / 8.0      # 1/sqrt(64)

_PROGRAM_CACHE = {}


def build_program():
    nc = bacc.Bacc("TRN2", target_bir_lowering=False, debug=False)

    x_d = nc.dram_tensor("x", [T, C], BF16, kind="ExternalInput").ap()
    wa_d = nc.dram_tensor("W_attn", [C, 3 * C], BF16, kind="ExternalInput").ap()
    ba_d = nc.dram_tensor("b_attn", [1, 2 * C], BF16, kind="ExternalInput").ap()
    wp_d = nc.dram_tensor("W_proj", [C, C], BF16, kind="ExternalInput").ap()
    bp_d = nc.dram_tensor("b_proj", [1, C], F32, kind="ExternalInput").ap()
    y_d = nc.dram_tensor("y", [T, C], BF16, kind="ExternalOutput").ap()

    with tile.TileContext(nc) as tc:
        _emit(nc, tc, x_d, wa_d, ba_d, wp_d, bp_d, y_d)
    nc.compile()
    return nc


def _emit(nc, tc, x_d, wa_d, ba_d, wp_d, bp_d, y_d):
    from contextlib import ExitStack

    ctx = ExitStack()
    with ctx:
        const_pool = ctx.enter_context(tc.tile_pool(name="consts", bufs=1))
        # ps_work holds the merged [128,1024] ST tiles (2 banks each);
        # ps_acc holds 1-bank accumulation tiles (qk/v/y/ot').
        ps_work = ctx.enter_context(tc.tile_pool(name="ps_work", bufs=2, space="PSUM"))
        ps_acc = ctx.enter_context(tc.tile_pool(name="ps_acc", bufs=2, space="PSUM"))

        # ---- input DMAs first: nothing can run until x/W land ----------
        phase_ctx = ExitStack()
        xsb_pool = phase_ctx.enter_context(tc.tile_pool(name="xsb", bufs=1, side="right"))
        w_pool = phase_ctx.enter_context(tc.tile_pool(name="w", bufs=1, side="right"))
        xsb = []
        for tp in range(TP):
            x_sb = xsb_pool.tile([128, C], BF16, name=f"x_sb_{tp}", tag=f"x_sb{tp}")
            xsb.append(x_sb)
        W = []
        for cc in range(CC):
            w_t = w_pool.tile([128, 3 * C], BF16, name=f"W_{cc}", tag=f"W{cc}")
            W.append(w_t)

        def _dma_x(tp):
            nc.sync.dma_start(xsb[tp][:], x_d[tp * 128 : (tp + 1) * 128, :])

        def _dma_w(part):
            for cc in range(CC):
                nc.sync.dma_start(
                    W[cc][:, part * C : (part + 1) * C],
                    wa_d[cc * 128 : (cc + 1) * 128, part * C : (part + 1) * C],
                )

        # order: x0,x1 (first transposes), W_v (v_chunk 0-3), x2,x3, W_q,
        # then the rest of x, then W_k — matches first-use order downstream
        _dma_x(0)
        _dma_x(1)
        _dma_w(2)
        _dma_x(2)
        _dma_x(3)
        _dma_w(0)
        for tp in range(4, TP):
            _dma_x(tp)
        _dma_w(1)

        # ---- constants -------------------------------------------------
        # PE clock (HAM) warm-up: dummy matmuls keep the PE busy during the
        # input DMA so the clock gate releases before the real work starts.
        wseed = const_pool.tile([128, 128], BF16, name="wseed")
        nc.gpsimd.memset(wseed[:], 0.5)
        for i in range(24):
            wm = ps_acc.tile([128, 128], F32, name=f"wm_{i}", tag="acc")
            nc.tensor.matmul(wm[:], wseed[:], wseed[:], start=True, stop=True)

        ident_f32 = const_pool.tile([128, 128], F32, name="ident_f32")
        nc.gpsimd.memset(ident_f32[:], 0.0)
        nc.gpsimd.affine_select(
            out=ident_f32[:], in_=ident_f32[:], compare_op=ALU.not_equal, fill=1.0,
            base=0, pattern=[[-1, 128]], channel_multiplier=1,
        )
        ident = const_pool.tile([128, 128], BF16, name="ident")
        nc.vector.tensor_copy(ident[:], ident_f32[:])

        # tri[j, i] = 1.0 if j <= i else 0.0   (keep lower-causal in [j,i] layout)
        tri_f32 = const_pool.tile([128, 128], F32, name="tri_f32")
        nc.gpsimd.memset(tri_f32[:], 1.0)
        nc.gpsimd.affine_select(
            out=tri_f32[:], in_=tri_f32[:], compare_op=ALU.is_ge, fill=0.0,
            base=0, pattern=[[1, 128]], channel_multiplier=-1,
        )
        tri = const_pool.tile([128, 128], BF16, name="tri")
        nc.vector.tensor_copy(tri[:], tri_f32[:])
        ones32 = const_pool.tile([128, 16], F32, name="ones32")
        nc.gpsimd.memset(ones32[:], 1.0)
        # ones row at partition 64, used for the tail-pair PE broadcast of Z
        ones65 = const_pool.tile([65, 64], F32, name="ones65")
        nc.gpsimd.memset(ones65[:], 1.0)

        # warm the exp table set early (hidden under input DMA)
        expwarm = const_pool.tile([1, 1], F32, name="expwarm")
        nc.scalar.activation(expwarm[:], ones32[0:1, 0:1], AF.Exp)

        # ---- phase A: build xT [768, 1024] -----------------------------
        xt_pool = phase_ctx.enter_context(tc.tile_pool(name="xt", bufs=1, side="right"))
        xT = []
        for cc in range(CC):
            t_ = xt_pool.tile([128, T], BF16, name=f"xT_{cc}", tag=f"xT{cc}")
            xT.append(t_)

        # bias loads go behind the big DMAs.
        # column layout of b_attn qk-part for per-partition bias add:
        # ba_col[p, cp] = b_attn[cp*128 + p]  (strided DMA, one-time, 3KB)
        bp_sb = const_pool.tile([1, C], F32, name="bp_sb")
        nc.sync.dma_start(bp_sb[:], bp_d[:, :])
        ba_col = const_pool.tile([128, QKCP], F32, name="ba_col")
        ba_colb = const_pool.tile([128, QKCP], BF16, name="ba_colb")
        nc.sync.dma_start(
            ba_colb[:],
            ba_d[:, 0 : QKCP * 128].rearrange("a (cp p) -> (a p) cp", p=128),
        )
        nc.vector.tensor_copy(ba_col[:], ba_colb[:])
        # broadcast effective output bias (b_v @ W_proj + b_proj) to all
        # partitions once; added in the y drain.
        bpb = const_pool.tile([128, C], F32, name="bpb")
        nc.gpsimd.partition_broadcast(bpb[:], bp_sb[:])

        def transpose_x(tp):
            for cc in range(CC):
                pt = ps_work.tile([128, 128], BF16, name=f"ps_xt_{tp}_{cc}", tag="ps")
                nc.tensor.transpose(pt[:], xsb[tp][:, cc * 128 : (cc + 1) * 128], ident[:])
                nc.vector.tensor_copy(xT[cc][:, tp * 128 : (tp + 1) * 128], pt[:])

        # tp 0-3 up front; tp 4-7 are emitted later so their wait on the late
        # x4-7 DMAs does not head-of-line block the v/qk chains on the PE FIFO
        for tp in range(4):
            transpose_x(tp)

        # ---- phase B: Vp then qkT (attention needs all Vp) -------------
        vp_pool = ctx.enter_context(tc.tile_pool(name="vp", bufs=1))
        Vp = []
        for tp in range(TP):
            t_ = vp_pool.tile([128, H * 65], BF16, name=f"Vp_{tp}", tag=f"Vp{tp}")
            Vp.append(t_)
            nc.vector.tensor_copy(
                t_.rearrange("p (h e) -> p h e", e=65)[:, :, 64:65],
                ones32[:, 0:H].rearrange("p (h e) -> p h e", e=1),
            )

        def v_half(tp, vc):  # v cols [1536+384*vc : 1536+384*(vc+1)]
            def emit():
                pv = ps_acc.tile([128, 384], F32, name=f"ps_v_{vc}_{tp}", tag="acc")
                for cc in range(CC):
                    nc.tensor.matmul(
                        pv[:],
                        xT[cc][:, tp * 128 : (tp + 1) * 128],
                        W[cc][:, 1536 + vc * 384 : 1536 + (vc + 1) * 384],
                        start=(cc == 0),
                        stop=(cc == CC - 1),
                    )
                # single strided drain: heads 6*vc .. 6*vc+5 in one copy
                nc.scalar.copy(
                    Vp[tp].rearrange("p (h e) -> p h e", e=65)[
                        :, 6 * vc : 6 * vc + 6, 0:64
                    ],
                    pv.rearrange("p (h d) -> p h d", d=64),
                )
            return emit

        def v_chunk(tp):
            v_half(tp, 0)()
            v_half(tp, 1)()

        qkt_pool = ctx.enter_context(tc.tile_pool(name="qkt", bufs=1))
        qkT = []
        for cp in range(QKCP):
            t_ = qkt_pool.tile([128, T], BF16, name=f"qkT_{cp}", tag=f"qkT{cp}")
            qkT.append(t_)

        def qk_chain(hp, tb, kk):
            def emit():
                cp = hp + 6 * kk
                pq = ps_acc.tile([128, 512], F32, name=f"ps_qk_{cp}_{tb}", tag="acc")
                for cc in range(CC):
                    nc.tensor.matmul(
                        pq[:],
                        W[cc][:, cp * 128 : (cp + 1) * 128],
                        xT[cc][:, tb * 512 : (tb + 1) * 512],
                        start=(cc == 0),
                        stop=(cc == CC - 1),
                    )
                # b_attn[c'] folded in as a per-partition scalar add
                nc.vector.tensor_scalar_add(
                    qkT[cp][:, tb * 512 : (tb + 1) * 512],
                    pq[:],
                    ba_col[:, cp : cp + 1],
                )
            return emit

        def qk_pair(hp, tbs=(0, 1)):
            for tb in tbs:
                for kk in range(2):
                    qk_chain(hp, tb, kk)()

        # ---- W_proj prefetch ------------------------------------------
        wp_pool = ctx.enter_context(tc.tile_pool(name="wp", bufs=1))
        Wp = []
        for cc in range(CC):
            w_t = wp_pool.tile([128, C], BF16, name=f"Wp_{cc}", tag=f"Wp{cc}")
            nc.sync.dma_start(w_t[:], wp_d[cc * 128 : (cc + 1) * 128, :])
            Wp.append(w_t)

        # ---- phase C/D: attention (ib-major) interleaved with proj -----
        atn_pool = ctx.enter_context(tc.tile_pool(name="atn", bufs=1))
        ATn = []
        for cp in range(CC):
            t_ = atn_pool.tile([128, T], BF16, name=f"ATn_{cp}", tag=f"ATn{cp}")
            ATn.append(t_)

        est_pool = ctx.enter_context(tc.tile_pool(name="est", bufs=10))
        nrm_pool = ctx.enter_context(tc.tile_pool(name="nrm", bufs=4))
        y_pool = ctx.enter_context(tc.tile_pool(name="ysb", bufs=2))

        def attention(hp, ib, tail=False, filler=()):
            # filler: PE work (proj chains) injected between jc iterations so
            # the tensor engine never idles while ScalarE runs exp (idle PE
            # re-arms the HAM clock gate and everything drops to half speed)
            filler = list(filler)
            qt = qkT[hp]
            kt = qkT[6 + hp]
            po = {}
            for s in range(2):  # head 2*hp + s
                po[s] = ps_acc.tile([65, 512], F32, name=f"ps_ot_{hp}_{ib}_{s}", tag="ot", bufs=2)
            njc = 4 * (ib + 1)
            fill_at = (
                {(i * njc) // len(filler) for i in range(len(filler))}
                if filler
                else set()
            )
            for jc in range(njc):
                r = jc - 4 * ib
                col0 = max(r, 0) * 128
                # merged pair tile: head A in cols [0:512], head B in [512:1024]
                pst = ps_work.tile([128, 1024], F32, name=f"ps_st_{hp}_{ib}_{jc}", tag="ps")
                for s in range(2):
                    r0 = 64 * s
                    # row-packed pair: s=0 uses PE rows 0-63, s=1 rows 64-127
                    nc.tensor.matmul(
                        pst[:, 512 * s + col0 : 512 * s + 512],
                        kt[r0 : r0 + 64, jc * 128 : (jc + 1) * 128],
                        qt[r0 : r0 + 64, ib * 512 + col0 : (ib + 1) * 512],
                        start=True,
                        stop=True,
                    )
                est = est_pool.tile([128, 1024], BF16, name=f"est_{hp}_{ib}_{jc}", tag="est")
                nc.scalar.activation(
                    est.rearrange("p (a f) -> p a f", a=2)[:, :, col0:512],
                    pst.rearrange("p (a f) -> p a f", a=2)[:, :, col0:512],
                    AF.Exp,
                    scale=SCALE,
                )
                if r >= 0:
                    for s in range(2):
                        # mask the diagonal 128x128 sub-block (multiplicative)
                        nc.vector.tensor_tensor(
                            est[:, 512 * s + col0 : 512 * s + col0 + 128],
                            est[:, 512 * s + col0 : 512 * s + col0 + 128],
                            tri[:],
                            op=ALU.mult,
                        )
                if filler and jc in fill_at:
                    # emitted between the exp issue and this jc's PV pair so
                    # the chain runs on the PE while ScalarE computes the exp
                    filler.pop(0)()
                for s in range(2):
                    h = 2 * hp + s
                    nc.tensor.matmul(
                        po[s][:, col0:512],
                        Vp[jc][:, h * 65 : h * 65 + 65],
                        est[:, 512 * s + col0 : 512 * s + 512],
                        start=(jc == 0),
                        stop=(jc == njc - 1),
                    )
            for f in filler:
                f()
            # normalization: ATn rows = OT'[0:64] / Z  (Z = row 64).
            # Copy OT' to SBUF right away so the PSUM slot frees in ~0.7us;
            # the (long-latency, off-critical-path) normalization then runs
            # entirely from SBUF: gpsimd scatters Z across 128 partitions,
            # fast-approx reciprocal at [128,4], gather back, broadcast.
            if tail:
                # Last pair: nothing follows, so the po banks can stay held.
                # Skip the slow gpsimd scatter/gather/broadcast chain: copy
                # the Z rows to SBUF, broadcast them across 64 partitions with
                # an fp32 PE matmul (ones outer product; PE is idle here and
                # this also keeps the HAM clock gate open), reciprocal at
                # [64,512], and drain ATn straight from PSUM with a fused
                # scalar_tensor_tensor multiply.
                zrow = [
                    nrm_pool.tile([65, 512], F32, name=f"ztl_{s}", tag=f"ztl{s}", bufs=1)
                    for s in range(2)
                ]
                zbps = ps_work.tile([128, 1024], F32, name="zb_tail", tag="ps")
                for s in range(2):
                    # scalar engine is idle at the tail; DVE is not
                    nc.scalar.copy(zrow[s][64:65, :], po[s][64:65, :])
                    nc.tensor.matmul(
                        zbps[0:64, 512 * s : 512 * s + 512],
                        ones65[64:65, :],
                        zrow[s][64:65, :],
                        start=True,
                        stop=True,
                    )
                # dummy matmuls bridge the norm window so the HAM clock gate
                # stays open for the final proj chains
                for i in range(8):
                    wmt = ps_acc.tile([128, 128], F32, name=f"wmt_{i}", tag="acc")
                    nc.tensor.matmul(wmt[:], wseed[:], wseed[:], start=True, stop=True)
                for s in range(2):
                    zbr = nrm_pool.tile([64, 512], F32, name=f"zbr_{s}", tag=f"zbr{s}", bufs=1)
                    nc.vector.reciprocal_approx_fast(
                        out=zbr[:], in_=zbps[0:64, 512 * s : 512 * s + 512]
                    )
                    nc.vector.scalar_tensor_tensor(
                        ATn[hp][64 * s : 64 * s + 64, ib * 512 : (ib + 1) * 512],
                        po[s][0:64, :],
                        0.0,
                        zbr[:],
                        op0=ALU.bypass,
                        op1=ALU.mult,
                    )
                return
            otu = [
                nrm_pool.tile([65, 512], F32, name=f"otu_{hp}_{ib}_{s}", tag=f"otu{s}")
                for s in range(2)
            ]
            for s in range(2):
                nc.vector.tensor_copy(otu[s][:], po[s][:, :])
            # batched Z chain for both heads: one reciprocal pass
            # (zs2[p, s*4+c] = Z_s[4p+c], gather inverts the same mapping)
            zs2 = nrm_pool.tile([128, 8], F32, name=f"zs_{hp}_{ib}", tag="zs")
            for s in range(2):
                nc.gpsimd.dma_start(zs2[:, 4 * s : 4 * s + 4], otu[s][64:65, :])
            zr2 = nrm_pool.tile([128, 8], F32, name=f"zr_{hp}_{ib}", tag="zr")
            nc.vector.reciprocal_approx_fast(out=zr2[:], in_=zs2[:])
            zinv = [
                nrm_pool.tile([1, 512], F32, name=f"zinv_{hp}_{ib}_{s}", tag=f"zinv{s}")
                for s in range(2)
            ]
            for s in range(2):
                nc.gpsimd.dma_start(zinv[s][:], zr2[:, 4 * s : 4 * s + 4])
            for s in range(2):
                zb = nrm_pool.tile([64, 512], F32, name=f"zb_{hp}_{ib}_{s}", tag=f"zb{s}")
                nc.gpsimd.partition_broadcast(zb[:], zinv[s][:])
                nc.vector.tensor_tensor(
                    ATn[hp][64 * s : 64 * s + 64, ib * 512 : (ib + 1) * 512],
                    otu[s][0:64, :],
                    zb[:],
                    op=ALU.mult,
                )

        # proj is emitted as per-(tp, oc) chains used as PE filler inside the
        # exp-bound attention phase. tp 0-3: full 6-link chains; tp 4-7 split
        # in two 3-link sessions with an SBUF accumulator so session A can run
        # before the last head-pairs' ATn exist.
        yacc_pool = ctx.enter_context(tc.tile_pool(name="yacc", bufs=1))
        y_acc = {}
        y_sbs = {}

        def _proj_links(tp, oc, cps, start, stop):
            py = ps_acc.tile([128, 384], F32, name=f"ps_y_{tp}_{oc}", tag="acc")
            for i, cp in enumerate(cps):
                nc.tensor.matmul(
                    py[:],
                    ATn[cp][:, tp * 128 : (tp + 1) * 128],
                    Wp[cp][:, oc * 384 : (oc + 1) * 384],
                    start=start and (i == 0),
                    stop=stop and (i == len(cps) - 1),
                )
            return py

        def _maybe_dma(tp):
            if y_sbs[tp]["done"] == 2:
                nc.sync.dma_start(y_d[tp * 128 : (tp + 1) * 128, :], y_sbs[tp]["t"][:])

        def proj_full(tp, oc):
            def emit():
                py = _proj_links(tp, oc, list(range(CC)), True, True)
                if tp not in y_sbs:
                    y_sbs[tp] = {
                        "t": y_pool.tile([128, C], BF16, name=f"y_sb_{tp}", tag="y_sb"),
                        "done": 0,
                    }
                nc.vector.scalar_tensor_tensor(
                    y_sbs[tp]["t"][:, oc * 384 : (oc + 1) * 384],
                    py[:],
                    0.0,
                    bpb[:, oc * 384 : (oc + 1) * 384],
                    op0=ALU.bypass,
                    op1=ALU.add,
                )
                y_sbs[tp]["done"] += 1
                _maybe_dma(tp)
            return emit

        def proj_sessA(tp, oc):
            def emit():
                py = _proj_links(tp, oc, [0, 1, 2], True, True)
                if tp not in y_acc:
                    y_acc[tp] = yacc_pool.tile(
                        [128, C], F32, name=f"y_acc_{tp}", tag=f"ya{tp}"
                    )
                # y_acc = partial + output bias (bias folded here once)
                nc.vector.scalar_tensor_tensor(
                    y_acc[tp][:, oc * 384 : (oc + 1) * 384],
                    py[:],
                    0.0,
                    bpb[:, oc * 384 : (oc + 1) * 384],
                    op0=ALU.bypass,
                    op1=ALU.add,
                )
            return emit

        def proj_sessB(tp, oc):
            def emit():
                py = _proj_links(tp, oc, [3, 4, 5], True, True)
                if tp not in y_sbs:
                    y_sbs[tp] = {
                        "t": y_pool.tile([128, C], BF16, name=f"y_sb_{tp}", tag="y_sb"),
                        "done": 0,
                    }
                nc.vector.scalar_tensor_tensor(
                    y_sbs[tp]["t"][:, oc * 384 : (oc + 1) * 384],
                    py[:],
                    0.0,
                    y_acc[tp][:, oc * 384 : (oc + 1) * 384],
                    op0=ALU.bypass,
                    op1=ALU.add,
                )
                y_sbs[tp]["done"] += 1
                _maybe_dma(tp)
            return emit

        # Emission schedule. attention(hp, 0) ib=0 needs qkT pair hp tb0,
        # Vp[0..3] vc=0 for hp<3 (vc=1 for hp>=3); attention(hp, 1) needs the
        # full qkT pair + Vp[0..7]. All remaining chain work (later qk chains,
        # v halves, transposes tp4-7, proj) is spread as PE filler inside the
        # exp-bound attention jc loops so the tensor engine never starves
        # (which would also re-arm the HAM throttle). Every filler is emitted
        # only where its inputs are already resident.
        for tp in range(4):
            v_half(tp, 0)()
        qk_pair(0, tbs=(0,))
        qk_pair(1, tbs=(0,))
        for tp in range(4, 8):
            transpose_x(tp)
        attention(0, 0, filler=[v_half(0, 1), v_half(1, 1), v_half(2, 1), v_half(3, 1)])
        attention(1, 0, filler=[qk_chain(2, 0, 0), qk_chain(2, 0, 1), v_half(4, 0), v_half(4, 1)])
        attention(2, 0, filler=[qk_chain(3, 0, 0), qk_chain(3, 0, 1), v_half(5, 0), v_half(5, 1)])
        attention(3, 0, filler=[qk_chain(4, 0, 0), qk_chain(4, 0, 1), v_half(6, 0), v_half(6, 1)])
        attention(4, 0, filler=[qk_chain(5, 0, 0), qk_chain(5, 0, 1), v_half(7, 0), v_half(7, 1)])
        attention(5, 0, filler=[qk_chain(0, 1, 0), qk_chain(0, 1, 1), qk_chain(1, 1, 0), qk_chain(1, 1, 1)])
        attention(0, 1, filler=[qk_chain(2, 1, 0), qk_chain(2, 1, 1), proj_full(0, 0), proj_full(0, 1)])
        attention(1, 1, filler=[qk_chain(3, 1, 0), qk_chain(3, 1, 1), proj_full(1, 0), proj_full(1, 1)])
        attention(2, 1, filler=[qk_chain(4, 1, 0), qk_chain(4, 1, 1), proj_full(2, 0), proj_full(2, 1)])
        attention(3, 1, filler=[qk_chain(5, 1, 0), qk_chain(5, 1, 1), proj_full(3, 0), proj_full(3, 1)])
        phase_ctx.close()  # release xt/xsb/w SBUF (all readers emitted)
        attention(
            4, 1,
            filler=[proj_sessA(4, 0), proj_sessA(4, 1), proj_sessA(5, 0), proj_sessA(5, 1)],
        )
        attention(
            5, 1, tail=True,
            filler=[proj_sessA(6, 0), proj_sessA(6, 1), proj_sessA(7, 0), proj_sessA(7, 1)],
        )
        for tp in range(4, 8):
            proj_sessB(tp, 0)()
            proj_sessB(tp, 1)()


def kernel(x, W_attn, b_attn, W_proj, b_proj, _trace=False, _trace_kwargs=None):
    import ml_dtypes

    bf16 = ml_dtypes.bfloat16
    x = np.asarray(x)
    W_attn = np.asarray(W_attn)
    b_attn = np.asarray(b_attn)
    W_proj = np.asarray(W_proj)
    b_proj = np.asarray(b_proj)
    # v-bias composes linearly through the projection (softmax rows sum to 1):
    # y = (softmax @ (xWv)) W_proj + (b_v W_proj + b_proj)
    bp_eff = (
        b_attn[2 * C :].astype(np.float64) @ W_proj.astype(np.float64)
        + b_proj.astype(np.float64)
    ).astype(np.float32)

    xb = np.ascontiguousarray(x.astype(bf16))
    W_attnb = np.ascontiguousarray(W_attn.astype(bf16))
    ba_qk = np.ascontiguousarray(b_attn[: 2 * C].astype(bf16)).reshape(1, 2 * C)
    W_projb = np.ascontiguousarray(W_proj.astype(bf16))
    bp_eff = np.ascontiguousarray(bp_eff).reshape(1, C)

    if "prog" not in _PROGRAM_CACHE:
        _PROGRAM_CACHE["prog"] = build_program()
    nc = _PROGRAM_CACHE["prog"]

    in_maps = [
        {
            "x": np.ascontiguousarray(xb[b]),
            "W_attn": W_attnb,
            "b_attn": ba_qk,
            "W_proj": W_projb,
            "b_proj": bp_eff,
        }
        for b in range(NCORES)
    ]
    res = run_bass_kernel_spmd(
        nc,
        in_maps,
        core_ids=list(range(NCORES)),
        trace=_trace,
        **(_trace_kwargs or {}),
    )
    out = np.stack(
        [res.results[b]["y"].astype(np.float32) for b in range(NCORES)], axis=0
    )
    if _trace:
        return out, res
    return out


if __name__ == "__main__":
    rng = np.random.default_rng(0)
    x = rng.standard_normal((NCORES, T, C)).astype(np.float32)
    W_attn = (rng.standard_normal((C, 3 * C)) * 0.02).astype(np.float32)
    b_attn = np.zeros(3 * C, np.float32)
    W_proj = (rng.standard_normal((C, C)) * 0.02).astype(np.float32)
    b_proj = np.zeros(C, np.float32)
    y = kernel(x=x, W_attn=W_attn, b_attn=b_attn, W_proj=W_proj, b_proj=b_proj)
    print("out", y.shape, y.dtype, np.abs(y).max())


# revision 36
# speedup vs baseline: 1.0347x; 1.0347x over previous
"""Causal self-attention kernel for TRN2 (8 NeuronCores, Bass/Tile).

Problem: B=8, T=1024, C=768, H=12, HD=64.
  qkv = x @ W_attn + b_attn ; causal softmax attention ; y = att_out @ W_proj + b_proj

Sharding: pure data-parallel over batch — core b computes batch element b.

Per-core dataflow (all matmuls bf16):
  xT   [768,1024]  = PE-transpose of x                      (lhsT/rhs source)
  qkT  [1536,1024] = (W_qk)^T-style projection: qkT[c',t] = sum_c W[c,c'] xT[c,t]
                     (+ b_qk folded in as per-partition scalar in the drain)
  V    [1024,768]  : V[t,c'] = sum_c xT[c,t] W_v[c,c']      (per-head Vp tiles with
                     a leading ones column -> PV matmul also produces Z row)
  per head h, i-block (512 cols):
     ST[j,i] = kT^T q  (K=64, causal-trimmed)   -> exp(0.125*ST) on ScalarE
     tri-mask on diagonal 128x128 sub-block (multiplicative, post-exp)
     OT'[0:64,:] = unnormalized attention out (transposed), OT'[64,:] = Z
     ATn[c,t] = OT'[0:64]/Z via fused scalar_tensor_tensor drain from PSUM
  y[t,:] = ATn^T-contraction with W_proj + bp_eff
  bias handling: b_qk in qkT drain; b_v and b_proj folded on the HOST into
  bp_eff = b_v @ W_proj + b_proj (v-bias composes linearly through proj since
  softmax rows sum to 1), added in the y drain from a broadcast tile.
"""

import numpy as np

import concourse.bass as bass
import concourse.mybir as mybir
import concourse.tile as tile
from concourse import bacc
from concourse.bass_utils import run_bass_kernel_spmd

F32 = mybir.dt.float32
BF16 = mybir.dt.bfloat16
AF = mybir.ActivationFunctionType
ALU = mybir.AluOpType

T, C, H, HD = 1024, 768, 12, 64
NCORES = 8
CC = C // 128          # 6 contraction chunks
TP = T // 128          # 8 t-chunks of 128
TB = T // 512          # 2 t-blocks of 512
QKCP = 2 * C // 128    # 12 qkT partition tiles
SCALE = 1.0 / 8.0      # 1/sqrt(64)

_PROGRAM_CACHE = {}


def build_program():
    nc = bacc.Bacc("TRN2", target_bir_lowering=False, debug=False)

    x_d = nc.dram_tensor("x", [T, C], BF16, kind="ExternalInput").ap()
    wa_d = nc.dram_tensor("W_attn", [C, 3 * C], BF16, kind="ExternalInput").ap()
    ba_d = nc.dram_tensor("b_attn", [1, 2 * C], BF16, kind="ExternalInput").ap()
    wp_d = nc.dram_tensor("W_proj", [C, C], BF16, kind="ExternalInput").ap()
    bp_d = nc.dram_tensor("b_proj", [1, C], F32, kind="ExternalInput").ap()
    y_d = nc.dram_tensor("y", [T, C], BF16, kind="ExternalOutput").ap()

    with tile.TileContext(nc) as tc:
        _emit(nc, tc, x_d, wa_d, ba_d, wp_d, bp_d, y_d)
    nc.compile()
    return nc


def _emit(nc, tc, x_d, wa_d, ba_d, wp_d, bp_d, y_d):
    from contextlib import ExitStack

    ctx = ExitStack()
    with ctx:
        const_pool = ctx.enter_context(tc.tile_pool(name="consts", bufs=1))
        # ps_work holds the merged [128,1024] ST tiles (2 banks each);
        # ps_acc holds 1-bank accumulation tiles (qk/v/y/ot').
        ps_work = ctx.enter_context(tc.tile_pool(name="ps_work", bufs=2, space="PSUM"))
        ps_acc = ctx.enter_context(tc.tile_pool(name="ps_acc", bufs=2, space="PSUM"))

        # ---- input DMAs first: nothing can run until x/W land ----------
        phase_ctx = ExitStack()
        xsb_pool = phase_ctx.enter_context(tc.tile_pool(name="xsb", bufs=1, side="right"))
        w_pool = phase_ctx.enter_context(tc.tile_pool(name="w", bufs=1, side="right"))
        xsb = []
        for tp in range(TP):
            x_sb = xsb_pool.tile([128, C], BF16, name=f"x_sb_{tp}", tag=f"x_sb{tp}")
            xsb.append(x_sb)
        W = []
        for cc in range(CC):
            w_t = w_pool.tile([128, 3 * C], BF16, name=f"W_{cc}", tag=f"W{cc}")
            W.append(w_t)

        def _dma_x(tp):
            nc.sync.dma_start(xsb[tp][:], x_d[tp * 128 : (tp + 1) * 128, :])

        def _dma_w(part):
            for cc in range(CC):
                nc.sync.dma_start(
                    W[cc][:, part * C : (part + 1) * C],
                    wa_d[cc * 128 : (cc + 1) * 128, part * C : (part + 1) * C],
                )

        # order: x0,x1 (first transposes), W_v (v_chunk 0-3), x2,x3, W_q,
        # then the rest of x, then W_k — matches first-use order downstream
        _dma_x(0)
        _dma_x(1)
        _dma_w(2)
        _dma_x(2)
        _dma_x(3)
        _dma_w(0)
        for tp in range(4, TP):
            _dma_x(tp)
        _dma_w(1)

        # ---- constants -------------------------------------------------
        # PE clock (HAM) warm-up: dummy matmuls keep the PE busy during the
        # input DMA so the clock gate releases before the real work starts.
        wseed = const_pool.tile([128, 128], BF16, name="wseed")
        nc.gpsimd.memset(wseed[:], 0.5)
        for i in range(24):
            wm = ps_acc.tile([128, 128], F32, name=f"wm_{i}", tag="acc")
            nc.tensor.matmul(wm[:], wseed[:], wseed[:], start=True, stop=True)

        ident_f32 = const_pool.tile([128, 128], F32, name="ident_f32")
        nc.gpsimd.memset(ident_f32[:], 0.0)
        nc.gpsimd.affine_select(
            out=ident_f32[:], in_=ident_f32[:], compare_op=ALU.not_equal, fill=1.0,
            base=0, pattern=[[-1, 128]], channel_multiplier=1,
        )
        ident = const_pool.tile([128, 128], BF16, name="ident")
        nc.vector.tensor_copy(ident[:], ident_f32[:])

        # tri[j, i] = 1.0 if j <= i else 0.0   (keep lower-causal in [j,i] layout)
        tri_f32 = const_pool.tile([128, 128], F32, name="tri_f32")
        nc.gpsimd.memset(tri_f32[:], 1.0)
        nc.gpsimd.affine_select(
            out=tri_f32[:], in_=tri_f32[:], compare_op=ALU.is_ge, fill=0.0,
            base=0, pattern=[[1, 128]], channel_multiplier=-1,
        )
        tri = const_pool.tile([128, 128], BF16, name="tri")
        nc.vector.tensor_copy(tri[:], tri_f32[:])
        ones32 = const_pool.tile([128, 16], F32, name="ones32")
        nc.gpsimd.memset(ones32[:], 1.0)
        # ones row at partition 64, used for the tail-pair PE broadcast of Z
        ones65 = const_pool.tile([65, 64], F32, name="ones65")
        nc.gpsimd.memset(ones65[:], 1.0)

        # warm the exp table set early (hidden under input DMA)
        expwarm = const_pool.tile([1, 1], F32, name="expwarm")
        nc.scalar.activation(expwarm[:], ones32[0:1, 0:1], AF.Exp)

        # ---- phase A: build xT [768, 1024] -----------------------------
        xt_pool = phase_ctx.enter_context(tc.tile_pool(name="xt", bufs=1, side="right"))
        xT = []
        for cc in range(CC):
            t_ = xt_pool.tile([128, T], BF16, name=f"xT_{cc}", tag=f"xT{cc}")
            xT.append(t_)

        # bias loads go behind the big DMAs.
        # column layout of b_attn qk-part for per-partition bias add:
        # ba_col[p, cp] = b_attn[cp*128 + p]  (strided DMA, one-time, 3KB)
        bp_sb = const_pool.tile([1, C], F32, name="bp_sb")
        nc.sync.dma_start(bp_sb[:], bp_d[:, :])
        ba_col = const_pool.tile([128, QKCP], F32, name="ba_col")
        ba_colb = const_pool.tile([128, QKCP], BF16, name="ba_colb")
        nc.sync.dma_start(
            ba_colb[:],
            ba_d[:, 0 : QKCP * 128].rearrange("a (cp p) -> (a p) cp", p=128),
        )
        nc.vector.tensor_copy(ba_col[:], ba_colb[:])
        # broadcast effective output bias (b_v @ W_proj + b_proj) to all
        # partitions once; added in the y drain.
        bpb = const_pool.tile([128, C], F32, name="bpb")
        nc.gpsimd.partition_broadcast(bpb[:], bp_sb[:])

        for tp in range(TP):
            for cc in range(CC):
                pt = ps_work.tile([128, 128], BF16, name=f"ps_xt_{tp}_{cc}", tag="ps")
                nc.tensor.transpose(pt[:], xsb[tp][:, cc * 128 : (cc + 1) * 128], ident[:])
                nc.vector.tensor_copy(xT[cc][:, tp * 128 : (tp + 1) * 128], pt[:])

        # ---- phase B: Vp then qkT (attention needs all Vp) -------------
        vp_pool = ctx.enter_context(tc.tile_pool(name="vp", bufs=1))
        Vp = []
        for tp in range(TP):
            t_ = vp_pool.tile([128, H * 65], BF16, name=f"Vp_{tp}", tag=f"Vp{tp}")
            Vp.append(t_)
            nc.vector.tensor_copy(
                t_.rearrange("p (h e) -> p h e", e=65)[:, :, 64:65],
                ones32[:, 0:H].rearrange("p (h e) -> p h e", e=1),
            )

        def v_half(tp, vc):  # v cols [1536+384*vc : 1536+384*(vc+1)]
            def emit():
                pv = ps_acc.tile([128, 384], F32, name=f"ps_v_{vc}_{tp}", tag="acc")
                for cc in range(CC):
                    nc.tensor.matmul(
                        pv[:],
                        xT[cc][:, tp * 128 : (tp + 1) * 128],
                        W[cc][:, 1536 + vc * 384 : 1536 + (vc + 1) * 384],
                        start=(cc == 0),
                        stop=(cc == CC - 1),
                    )
                # single strided drain: heads 6*vc .. 6*vc+5 in one copy
                nc.scalar.copy(
                    Vp[tp].rearrange("p (h e) -> p h e", e=65)[
                        :, 6 * vc : 6 * vc + 6, 0:64
                    ],
                    pv.rearrange("p (h d) -> p h d", d=64),
                )
            return emit

        def v_chunk(tp):
            v_half(tp, 0)()
            v_half(tp, 1)()

        qkt_pool = ctx.enter_context(tc.tile_pool(name="qkt", bufs=1))
        qkT = []
        for cp in range(QKCP):
            t_ = qkt_pool.tile([128, T], BF16, name=f"qkT_{cp}", tag=f"qkT{cp}")
            qkT.append(t_)

        def qk_chain(hp, tb, kk):
            def emit():
                cp = hp + 6 * kk
                pq = ps_acc.tile([128, 512], F32, name=f"ps_qk_{cp}_{tb}", tag="acc")
                for cc in range(CC):
                    nc.tensor.matmul(
                        pq[:],
                        W[cc][:, cp * 128 : (cp + 1) * 128],
                        xT[cc][:, tb * 512 : (tb + 1) * 512],
                        start=(cc == 0),
                        stop=(cc == CC - 1),
                    )
                # b_attn[c'] folded in as a per-partition scalar add
                nc.vector.tensor_scalar_add(
                    qkT[cp][:, tb * 512 : (tb + 1) * 512],
                    pq[:],
                    ba_col[:, cp : cp + 1],
                )
            return emit

        def qk_pair(hp, tbs=(0, 1)):
            for tb in tbs:
                for kk in range(2):
                    qk_chain(hp, tb, kk)()

        # ---- W_proj prefetch ------------------------------------------
        wp_pool = ctx.enter_context(tc.tile_pool(name="wp", bufs=1))
        Wp = []
        for cc in range(CC):
            w_t = wp_pool.tile([128, C], BF16, name=f"Wp_{cc}", tag=f"Wp{cc}")
            nc.sync.dma_start(w_t[:], wp_d[cc * 128 : (cc + 1) * 128, :])
            Wp.append(w_t)

        # ---- phase C/D: attention (ib-major) interleaved with proj -----
        atn_pool = ctx.enter_context(tc.tile_pool(name="atn", bufs=1))
        ATn = []
        for cp in range(CC):
            t_ = atn_pool.tile([128, T], BF16, name=f"ATn_{cp}", tag=f"ATn{cp}")
            ATn.append(t_)

        est_pool = ctx.enter_context(tc.tile_pool(name="est", bufs=10))
        nrm_pool = ctx.enter_context(tc.tile_pool(name="nrm", bufs=4))
        y_pool = ctx.enter_context(tc.tile_pool(name="ysb", bufs=2))

        def attention(hp, ib, tail=False, filler=()):
            # filler: PE work (proj chains) injected between jc iterations so
            # the tensor engine never idles while ScalarE runs exp (idle PE
            # re-arms the HAM clock gate and everything drops to half speed)
            filler = list(filler)
            qt = qkT[hp]
            kt = qkT[6 + hp]
            po = {}
            for s in range(2):  # head 2*hp + s
                po[s] = ps_acc.tile([65, 512], F32, name=f"ps_ot_{hp}_{ib}_{s}", tag="ot", bufs=2)
            njc = 4 * (ib + 1)
            fill_at = (
                {(i * njc) // len(filler) for i in range(len(filler))}
                if filler
                else set()
            )
            for jc in range(njc):
                r = jc - 4 * ib
                col0 = max(r, 0) * 128
                # merged pair tile: head A in cols [0:512], head B in [512:1024]
                pst = ps_work.tile([128, 1024], F32, name=f"ps_st_{hp}_{ib}_{jc}", tag="ps")
                for s in range(2):
                    r0 = 64 * s
                    # row-packed pair: s=0 uses PE rows 0-63, s=1 rows 64-127
                    nc.tensor.matmul(
                        pst[:, 512 * s + col0 : 512 * s + 512],
                        kt[r0 : r0 + 64, jc * 128 : (jc + 1) * 128],
                        qt[r0 : r0 + 64, ib * 512 + col0 : (ib + 1) * 512],
                        start=True,
                        stop=True,
                    )
                est = est_pool.tile([128, 1024], BF16, name=f"est_{hp}_{ib}_{jc}", tag="est")
                nc.scalar.activation(
                    est.rearrange("p (a f) -> p a f", a=2)[:, :, col0:512],
                    pst.rearrange("p (a f) -> p a f", a=2)[:, :, col0:512],
                    AF.Exp,
                    scale=SCALE,
                )
                if r >= 0:
                    for s in range(2):
                        # mask the diagonal 128x128 sub-block (multiplicative)
                        nc.vector.tensor_tensor(
                            est[:, 512 * s + col0 : 512 * s + col0 + 128],
                            est[:, 512 * s + col0 : 512 * s + col0 + 128],
                            tri[:],
                            op=ALU.mult,
                        )
                if filler and jc in fill_at:
                    # emitted between the exp issue and this jc's PV pair so
                    # the chain runs on the PE while ScalarE computes the exp
                    filler.pop(0)()
                for s in range(2):
                    h = 2 * hp + s
                    nc.tensor.matmul(
                        po[s][:, col0:512],
                        Vp[jc][:, h * 65 : h * 65 + 65],
                        est[:, 512 * s + col0 : 512 * s + 512],
                        start=(jc == 0),
                        stop=(jc == njc - 1),
                    )
            for f in filler:
                f()
            # normalization: ATn rows = OT'[0:64] / Z  (Z = row 64).
            # Copy OT' to SBUF right away so the PSUM slot frees in ~0.7us;
            # the (long-latency, off-critical-path) normalization then runs
            # entirely from SBUF: gpsimd scatters Z across 128 partitions,
            # fast-approx reciprocal at [128,4], gather back, broadcast.
            if tail:
                # Last pair: nothing follows, so the po banks can stay held.
                # Skip the slow gpsimd scatter/gather/broadcast chain: copy
                # the Z rows to SBUF, broadcast them across 64 partitions with
                # an fp32 PE matmul (ones outer product; PE is idle here and
                # this also keeps the HAM clock gate open), reciprocal at
                # [64,512], and drain ATn straight from PSUM with a fused
                # scalar_tensor_tensor multiply.
                zrow = [
                    nrm_pool.tile([65, 512], F32, name=f"ztl_{s}", tag=f"ztl{s}", bufs=1)
                    for s in range(2)
                ]
                zbps = ps_work.tile([128, 1024], F32, name="zb_tail", tag="ps")
                for s in range(2):
                    # scalar engine is idle at the tail; DVE is not
                    nc.scalar.copy(zrow[s][64:65, :], po[s][64:65, :])
                    nc.tensor.matmul(
                        zbps[0:64, 512 * s : 512 * s + 512],
                        ones65[64:65, :],
                        zrow[s][64:65, :],
                        start=True,
                        stop=True,
                    )
                # dummy matmuls bridge the norm window so the HAM clock gate
                # stays open for the final proj chains
                for i in range(8):
                    wmt = ps_acc.tile([128, 128], F32, name=f"wmt_{i}", tag="acc")
                    nc.tensor.matmul(wmt[:], wseed[:], wseed[:], start=True, stop=True)
                for s in range(2):
                    zbr = nrm_pool.tile([64, 512], F32, name=f"zbr_{s}", tag=f"zbr{s}", bufs=1)
                    nc.vector.reciprocal_approx_fast(
                        out=zbr[:], in_=zbps[0:64, 512 * s : 512 * s + 512]
                    )
                    nc.vector.scalar_tensor_tensor(
                        ATn[hp][64 * s : 64 * s + 64, ib * 512 : (ib + 1) * 512],
                        po[s][0:64, :],
                        0.0,
                        zbr[:],
                        op0=ALU.bypass,
                        op1=ALU.mult,
                    )
                return
            otu = [
                nrm_pool.tile([65, 512], F32, name=f"otu_{hp}_{ib}_{s}", tag=f"otu{s}")
                for s in range(2)
            ]
            for s in range(2):
                nc.vector.tensor_copy(otu[s][:], po[s][:, :])
            # batched Z chain for both heads: one reciprocal pass
            # (zs2[p, s*4+c] = Z_s[4p+c], gather inverts the same mapping)
            zs2 = nrm_pool.tile([128, 8], F32, name=f"zs_{hp}_{ib}", tag="zs")
            for s in range(2):
                nc.gpsimd.dma_start(zs2[:, 4 * s : 4 * s + 4], otu[s][64:65, :])
            zr2 = nrm_pool.tile([128, 8], F32, name=f"zr_{hp}_{ib}", tag="zr")
            nc.vector.reciprocal_approx_fast(out=zr2[:], in_=zs2[:])
            zinv = [
                nrm_pool.tile([1, 512], F32, name=f"zinv_{hp}_{ib}_{s}", tag=f"zinv{s}")
                for s in range(2)
            ]
            for s in range(2):
                nc.gpsimd.dma_start(zinv[s][:], zr2[:, 4 * s : 4 * s + 4])
            for s in range(2):
                zb = nrm_pool.tile([64, 512], F32, name=f"zb_{hp}_{ib}_{s}", tag=f"zb{s}")
                nc.gpsimd.partition_broadcast(zb[:], zinv[s][:])
                nc.vector.tensor_tensor(
                    ATn[hp][64 * s : 64 * s + 64, ib * 512 : (ib + 1) * 512],
                    otu[s][0:64, :],
                    zb[:],
                    op=ALU.mult,
                )

        # proj is emitted as per-(tp, oc) chains used as PE filler inside the
        # exp-bound attention phase. tp 0-3: full 6-link chains; tp 4-7 split
        # in two 3-link sessions with an SBUF accumulator so session A can run
        # before the last head-pairs' ATn exist.
        yacc_pool = ctx.enter_context(tc.tile_pool(name="yacc", bufs=1))
        y_acc = {}
        y_sbs = {}

        def _proj_links(tp, oc, cps, start, stop):
            py = ps_acc.tile([128, 384], F32, name=f"ps_y_{tp}_{oc}", tag="acc")
            for i, cp in enumerate(cps):
                nc.tensor.matmul(
                    py[:],
                    ATn[cp][:, tp * 128 : (tp + 1) * 128],
                    Wp[cp][:, oc * 384 : (oc + 1) * 384],
                    start=start and (i == 0),
                    stop=stop and (i == len(cps) - 1),
                )
            return py

        def _maybe_dma(tp):
            if y_sbs[tp]["done"] == 2:
                nc.sync.dma_start(y_d[tp * 128 : (tp + 1) * 128, :], y_sbs[tp]["t"][:])

        def proj_full(tp, oc):
            def emit():
                py = _proj_links(tp, oc, list(range(CC)), True, True)
                if tp not in y_sbs:
                    y_sbs[tp] = {
                        "t": y_pool.tile([128, C], BF16, name=f"y_sb_{tp}", tag="y_sb"),
                        "done": 0,
                    }
                nc.vector.scalar_tensor_tensor(
                    y_sbs[tp]["t"][:, oc * 384 : (oc + 1) * 384],
                    py[:],
                    0.0,
                    bpb[:, oc * 384 : (oc + 1) * 384],
                    op0=ALU.bypass,
                    op1=ALU.add,
                )
                y_sbs[tp]["done"] += 1
                _maybe_dma(tp)
            return emit

        def proj_sessA(tp, oc):
            def emit():
                py = _proj_links(tp, oc, [0, 1, 2], True, True)
                if tp not in y_acc:
                    y_acc[tp] = yacc_pool.tile(
                        [128, C], F32, name=f"y_acc_{tp}", tag=f"ya{tp}"
                    )
                # y_acc = partial + output bias (bias folded here once)
                nc.vector.scalar_tensor_tensor(
                    y_acc[tp][:, oc * 384 : (oc + 1) * 384],
                    py[:],
                    0.0,
                    bpb[:, oc * 384 : (oc + 1) * 384],
                    op0=ALU.bypass,
                    op1=ALU.add,
                )
            return emit

        def proj_sessB(tp, oc):
            def emit():
                py = _proj_links(tp, oc, [3, 4, 5], True, True)
                if tp not in y_sbs:
                    y_sbs[tp] = {
                        "t": y_pool.tile([128, C], BF16, name=f"y_sb_{tp}", tag="y_sb"),
                        "done": 0,
                    }
                nc.vector.scalar_tensor_tensor(
                    y_sbs[tp]["t"][:, oc * 384 : (oc + 1) * 384],
                    py[:],
                    0.0,
                    y_acc[tp][:, oc * 384 : (oc + 1) * 384],
                    op0=ALU.bypass,
                    op1=ALU.add,
                )
                y_sbs[tp]["done"] += 1
                _maybe_dma(tp)
            return emit

        # Emission schedule. attention(hp, 0) ib=0 needs qkT pair hp tb0,
        # Vp[0..3] vc=0 for hp<3 (vc=1 for hp>=3); attention(hp, 1) needs the
        # full qkT pair + Vp[0..7]. All remaining chain work (later qk chains,
        # v halves, transposes tp4-7, proj) is spread as PE filler inside the
        # exp-bound attention jc loops so the tensor engine never starves
        # (which would also re-arm the HAM throttle). Every filler is emitted
        # only where its inputs are already resident.
        for tp in range(4):
            v_chunk(tp)
        qk_pair(0, tbs=(0,))
        qk_pair(1, tbs=(0,))
        attention(0, 0, filler=[v_half(4, 0), v_half(4, 1), v_half(5, 0), v_half(5, 1)])
        attention(1, 0, filler=[qk_chain(2, 0, 0), qk_chain(2, 0, 1), v_half(6, 0), v_half(6, 1)])
        attention(2, 0, filler=[qk_chain(3, 0, 0), qk_chain(3, 0, 1), v_half(7, 0), v_half(7, 1)])
        attention(3, 0, filler=[qk_chain(4, 0, 0), qk_chain(4, 0, 1), qk_chain(0, 1, 0), qk_chain(0, 1, 1)])
        attention(4, 0, filler=[qk_chain(5, 0, 0), qk_chain(5, 0, 1), qk_chain(1, 1, 0), qk_chain(1, 1, 1)])
        attention(5, 0, filler=[qk_chain(2, 1, 0), qk_chain(2, 1, 1), qk_chain(3, 1, 0), qk_chain(3, 1, 1)])
        attention(0, 1, filler=[qk_chain(4, 1, 0), qk_chain(4, 1, 1), proj_full(0, 0), proj_full(0, 1)])
        attention(1, 1, filler=[qk_chain(5, 1, 0), qk_chain(5, 1, 1), proj_full(1, 0), proj_full(1, 1)])
        phase_ctx.close()  # release xt/xsb/w SBUF (all readers emitted)
        attention(2, 1, filler=[proj_full(2, 0), proj_full(2, 1)])
        attention(3, 1, filler=[proj_full(3, 0), proj_full(3, 1)])
        attention(
            4, 1,
            filler=[proj_sessA(4, 0), proj_sessA(4, 1), proj_sessA(5, 0), proj_sessA(5, 1)],
        )
        attention(
            5, 1, tail=True,
            filler=[proj_sessA(6, 0), proj_sessA(6, 1), proj_sessA(7, 0), proj_sessA(7, 1)],
        )
        for tp in range(4, 8):
            proj_sessB(tp, 0)()
            proj_sessB(tp, 1)()


def kernel(x, W_attn, b_attn, W_proj, b_proj, _trace=False, _trace_kwargs=None):
    import ml_dtypes

    bf16 = ml_dtypes.bfloat16
    x = np.asarray(x)
    W_attn = np.asarray(W_attn)
    b_attn = np.asarray(b_attn)
    W_proj = np.asarray(W_proj)
    b_proj = np.asarray(b_proj)
    # v-bias composes linearly through the projection (softmax rows sum to 1):
    # y = (softmax @ (xWv)) W_proj + (b_v W_proj + b_proj)
    bp_eff = (
        b_attn[2 * C :].astype(np.float64) @ W_proj.astype(np.float64)
        + b_proj.astype(np.float64)
    ).astype(np.float32)

    xb = np.ascontiguousarray(x.astype(bf16))
    W_attnb = np.ascontiguousarray(W_attn.astype(bf16))
    ba_qk = np.ascontiguousarray(b_attn[: 2 * C].astype(bf16)).reshape(1, 2 * C)
    W_projb = np.ascontiguousarray(W_proj.astype(bf16))
    bp_eff = np.ascontiguousarray(bp_eff).reshape(1, C)

    if "prog" not in _PROGRAM_CACHE:
        _PROGRAM_CACHE["prog"] = build_program()
    nc = _PROGRAM_CACHE["prog"]

    in_maps = [
        {
            "x": np.ascontiguousarray(xb[b]),
            "W_attn": W_attnb,
            "b_attn": ba_qk,
            "W_proj": W_projb,
            "b_proj": bp_eff,
        }
        for b in range(NCORES)
    ]
    res = run_bass_kernel_spmd(
        nc,
        in_maps,
        core_ids=list(range(NCORES)),
        trace=_trace,
        **(_trace_kwargs or {}),
    )
    out = np.stack(
        [res.results[b]["y"].astype(np.float32) for b in range(NCORES)], axis=0
    )
    if _trace:
        return out, res
    return out


if __name__ == "__main__":
    rng = np.random.default_rng(0)
    x = rng.standard_normal((NCORES, T, C)).astype(np.float32)
    W_attn = (rng.standard_normal((C, 3 * C)) * 0.02).astype(np.float32)
    b_attn = np.zeros(3 * C, np.float32)
    W_proj = (rng.standard_normal((C, C)) * 0.02).astype(np.float32)
    b_proj = np.zeros(C, np.float32)
    y = kernel(x=x, W_attn=W_attn, b_attn=b_attn, W_proj=W_proj, b_proj=b_proj)
    print("out", y.shape, y.dtype, np.abs(y).max())


# revision 37
# speedup vs baseline: 1.0362x; 1.0014x over previous
"""Causal self-attention kernel for TRN2 (8 NeuronCores, Bass/Tile).

Problem: B=8, T=1024, C=768, H=12, HD=64.
  qkv = x @ W_attn + b_attn ; causal softmax attention ; y = att_out @ W_proj + b_proj

Sharding: pure data-parallel over batch — core b computes batch element b.

Per-core dataflow (all matmuls bf16):
  xT   [768,1024]  = PE-transpose of x                      (lhsT/rhs source)
  qkT  [1536,1024] = (W_qk)^T-style projection: qkT[c',t] = sum_c W[c,c'] xT[c,t]
                     (+ b_qk folded in as per-partition scalar in the drain)
  V    [1024,768]  : V[t,c'] = sum_c xT[c,t] W_v[c,c']      (per-head Vp tiles with
                     a leading ones column -> PV matmul also produces Z row)
  per head h, i-block (512 cols):
     ST[j,i] = kT^T q  (K=64, causal-trimmed)   -> exp(0.125*ST) on ScalarE
     tri-mask on diagonal 128x128 sub-block (multiplicative, post-exp)
     OT'[0:64,:] = unnormalized attention out (transposed), OT'[64,:] = Z
     ATn[c,t] = OT'[0:64]/Z via fused scalar_tensor_tensor drain from PSUM
  y[t,:] = ATn^T-contraction with W_proj + bp_eff
  bias handling: b_qk in qkT drain; b_v and b_proj folded on the HOST into
  bp_eff = b_v @ W_proj + b_proj (v-bias composes linearly through proj since
  softmax rows sum to 1), added in the y drain from a broadcast tile.
"""

import numpy as np

import concourse.bass as bass
import concourse.mybir as mybir
import concourse.tile as tile
from concourse import bacc
from concourse.bass_utils import run_bass_kernel_spmd

F32 = mybir.dt.float32
BF16 = mybir.dt.bfloat16
AF = mybir.ActivationFunctionType
ALU = mybir.AluOpType

T, C, H, HD = 1024, 768, 12, 64
NCORES = 8
CC = C // 128          # 6 contraction chunks
TP = T // 128          # 8 t-chunks of 128
TB = T // 512          # 2 t-blocks of 512
QKCP = 2 * C // 128    # 12 qkT partition tiles
SCALE = 1.0 / 8.0      # 1/sqrt(64)

_PROGRAM_CACHE = {}


def build_program():
    nc = bacc.Bacc("TRN2", target_bir_lowering=False, debug=False)

    x_d = nc.dram_tensor("x", [T, C], BF16, kind="ExternalInput").ap()
    wa_d = nc.dram_tensor("W_attn", [C, 3 * C], BF16, kind="ExternalInput").ap()
    ba_d = nc.dram_tensor("b_attn", [1, 2 * C], BF16, kind="ExternalInput").ap()
    wp_d = nc.dram_tensor("W_proj", [C, C], BF16, kind="ExternalInput").ap()
    bp_d = nc.dram_tensor("b_proj", [1, C], F32, kind="ExternalInput").ap()
    y_d = nc.dram_tensor("y", [T, C], BF16, kind="ExternalOutput").ap()

    with tile.TileContext(nc) as tc:
        _emit(nc, tc, x_d, wa_d, ba_d, wp_d, bp_d, y_d)
    nc.compile()
    return nc


def _emit(nc, tc, x_d, wa_d, ba_d, wp_d, bp_d, y_d):
    from contextlib import ExitStack

    ctx = ExitStack()
    with ctx:
        const_pool = ctx.enter_context(tc.tile_pool(name="consts", bufs=1))
        # ps_work holds the merged [128,1024] ST tiles (2 banks each);
        # ps_acc holds 1-bank accumulation tiles (qk/v/y/ot').
        ps_work = ctx.enter_context(tc.tile_pool(name="ps_work", bufs=2, space="PSUM"))
        ps_acc = ctx.enter_context(tc.tile_pool(name="ps_acc", bufs=2, space="PSUM"))

        # ---- input DMAs first: nothing can run until x/W land ----------
        phase_ctx = ExitStack()
        xsb_pool = phase_ctx.enter_context(tc.tile_pool(name="xsb", bufs=1, side="right"))
        w_pool = phase_ctx.enter_context(tc.tile_pool(name="w", bufs=1, side="right"))
        xsb = []
        for tp in range(TP):
            x_sb = xsb_pool.tile([128, C], BF16, name=f"x_sb_{tp}", tag=f"x_sb{tp}")
            xsb.append(x_sb)
        W = []
        for cc in range(CC):
            w_t = w_pool.tile([128, 3 * C], BF16, name=f"W_{cc}", tag=f"W{cc}")
            W.append(w_t)

        def _dma_x(tp):
            nc.sync.dma_start(xsb[tp][:], x_d[tp * 128 : (tp + 1) * 128, :])

        def _dma_w(part):
            for cc in range(CC):
                nc.sync.dma_start(
                    W[cc][:, part * C : (part + 1) * C],
                    wa_d[cc * 128 : (cc + 1) * 128, part * C : (part + 1) * C],
                )

        # order: x0,x1 (first transposes), W_v (v_chunk 0-3), x2,x3, W_q,
        # then the rest of x, then W_k — matches first-use order downstream
        _dma_x(0)
        _dma_x(1)
        _dma_w(2)
        _dma_x(2)
        _dma_x(3)
        _dma_w(0)
        for tp in range(4, TP):
            _dma_x(tp)
        _dma_w(1)

        # ---- constants -------------------------------------------------
        # PE clock (HAM) warm-up: dummy matmuls keep the PE busy during the
        # input DMA so the clock gate releases before the real work starts.
        wseed = const_pool.tile([128, 128], BF16, name="wseed")
        nc.gpsimd.memset(wseed[:], 0.5)
        for i in range(24):
            wm = ps_acc.tile([128, 128], F32, name=f"wm_{i}", tag="acc")
            nc.tensor.matmul(wm[:], wseed[:], wseed[:], start=True, stop=True)

        ident_f32 = const_pool.tile([128, 128], F32, name="ident_f32")
        nc.gpsimd.memset(ident_f32[:], 0.0)
        nc.gpsimd.affine_select(
            out=ident_f32[:], in_=ident_f32[:], compare_op=ALU.not_equal, fill=1.0,
            base=0, pattern=[[-1, 128]], channel_multiplier=1,
        )
        ident = const_pool.tile([128, 128], BF16, name="ident")
        nc.vector.tensor_copy(ident[:], ident_f32[:])

        # tri[j, i] = 1.0 if j <= i else 0.0   (keep lower-causal in [j,i] layout)
        tri_f32 = const_pool.tile([128, 128], F32, name="tri_f32")
        nc.gpsimd.memset(tri_f32[:], 1.0)
        nc.gpsimd.affine_select(
            out=tri_f32[:], in_=tri_f32[:], compare_op=ALU.is_ge, fill=0.0,
            base=0, pattern=[[1, 128]], channel_multiplier=-1,
        )
        tri = const_pool.tile([128, 128], BF16, name="tri")
        nc.vector.tensor_copy(tri[:], tri_f32[:])
        ones32 = const_pool.tile([128, 16], F32, name="ones32")
        nc.gpsimd.memset(ones32[:], 1.0)
        # ones row at partition 64, used for the tail-pair PE broadcast of Z
        ones65 = const_pool.tile([65, 64], F32, name="ones65")
        nc.gpsimd.memset(ones65[:], 1.0)

        # warm the exp table set early (hidden under input DMA)
        expwarm = const_pool.tile([1, 1], F32, name="expwarm")
        nc.scalar.activation(expwarm[:], ones32[0:1, 0:1], AF.Exp)

        # ---- phase A: build xT [768, 1024] -----------------------------
        xt_pool = phase_ctx.enter_context(tc.tile_pool(name="xt", bufs=1, side="right"))
        xT = []
        for cc in range(CC):
            t_ = xt_pool.tile([128, T], BF16, name=f"xT_{cc}", tag=f"xT{cc}")
            xT.append(t_)

        # bias loads go behind the big DMAs.
        # column layout of b_attn qk-part for per-partition bias add:
        # ba_col[p, cp] = b_attn[cp*128 + p]  (strided DMA, one-time, 3KB)
        bp_sb = const_pool.tile([1, C], F32, name="bp_sb")
        nc.sync.dma_start(bp_sb[:], bp_d[:, :])
        ba_col = const_pool.tile([128, QKCP], F32, name="ba_col")
        ba_colb = const_pool.tile([128, QKCP], BF16, name="ba_colb")
        nc.sync.dma_start(
            ba_colb[:],
            ba_d[:, 0 : QKCP * 128].rearrange("a (cp p) -> (a p) cp", p=128),
        )
        nc.vector.tensor_copy(ba_col[:], ba_colb[:])
        # broadcast effective output bias (b_v @ W_proj + b_proj) to all
        # partitions once; added in the y drain.
        bpb = const_pool.tile([128, C], F32, name="bpb")
        nc.gpsimd.partition_broadcast(bpb[:], bp_sb[:])

        def transpose_x(tp):
            for cc in range(CC):
                pt = ps_work.tile([128, 128], BF16, name=f"ps_xt_{tp}_{cc}", tag="ps")
                nc.tensor.transpose(pt[:], xsb[tp][:, cc * 128 : (cc + 1) * 128], ident[:])
                nc.vector.tensor_copy(xT[cc][:, tp * 128 : (tp + 1) * 128], pt[:])

        for tp in range(4):
            transpose_x(tp)

        # ---- phase B: Vp then qkT (attention needs all Vp) -------------
        vp_pool = ctx.enter_context(tc.tile_pool(name="vp", bufs=1))
        Vp = []
        for tp in range(TP):
            t_ = vp_pool.tile([128, H * 65], BF16, name=f"Vp_{tp}", tag=f"Vp{tp}")
            Vp.append(t_)
            nc.vector.tensor_copy(
                t_.rearrange("p (h e) -> p h e", e=65)[:, :, 64:65],
                ones32[:, 0:H].rearrange("p (h e) -> p h e", e=1),
            )

        def v_half(tp, vc):  # v cols [1536+384*vc : 1536+384*(vc+1)]
            def emit():
                pv = ps_acc.tile([128, 384], F32, name=f"ps_v_{vc}_{tp}", tag="acc")
                for cc in range(CC):
                    nc.tensor.matmul(
                        pv[:],
                        xT[cc][:, tp * 128 : (tp + 1) * 128],
                        W[cc][:, 1536 + vc * 384 : 1536 + (vc + 1) * 384],
                        start=(cc == 0),
                        stop=(cc == CC - 1),
                    )
                # single strided drain: heads 6*vc .. 6*vc+5 in one copy
                nc.scalar.copy(
                    Vp[tp].rearrange("p (h e) -> p h e", e=65)[
                        :, 6 * vc : 6 * vc + 6, 0:64
                    ],
                    pv.rearrange("p (h d) -> p h d", d=64),
                )
            return emit

        def v_chunk(tp):
            v_half(tp, 0)()
            v_half(tp, 1)()

        qkt_pool = ctx.enter_context(tc.tile_pool(name="qkt", bufs=1))
        qkT = []
        for cp in range(QKCP):
            t_ = qkt_pool.tile([128, T], BF16, name=f"qkT_{cp}", tag=f"qkT{cp}")
            qkT.append(t_)

        def qk_chain(hp, tb, kk):
            def emit():
                cp = hp + 6 * kk
                pq = ps_acc.tile([128, 512], F32, name=f"ps_qk_{cp}_{tb}", tag="acc")
                for cc in range(CC):
                    nc.tensor.matmul(
                        pq[:],
                        W[cc][:, cp * 128 : (cp + 1) * 128],
                        xT[cc][:, tb * 512 : (tb + 1) * 512],
                        start=(cc == 0),
                        stop=(cc == CC - 1),
                    )
                # b_attn[c'] folded in as a per-partition scalar add
                nc.vector.tensor_scalar_add(
                    qkT[cp][:, tb * 512 : (tb + 1) * 512],
                    pq[:],
                    ba_col[:, cp : cp + 1],
                )
            return emit

        def qk_pair(hp, tbs=(0, 1)):
            for tb in tbs:
                for kk in range(2):
                    qk_chain(hp, tb, kk)()

        # ---- W_proj prefetch ------------------------------------------
        wp_pool = ctx.enter_context(tc.tile_pool(name="wp", bufs=1))
        Wp = []
        for cc in range(CC):
            w_t = wp_pool.tile([128, C], BF16, name=f"Wp_{cc}", tag=f"Wp{cc}")
            nc.sync.dma_start(w_t[:], wp_d[cc * 128 : (cc + 1) * 128, :])
            Wp.append(w_t)

        # ---- phase C/D: attention (ib-major) interleaved with proj -----
        atn_pool = ctx.enter_context(tc.tile_pool(name="atn", bufs=1))
        ATn = []
        for cp in range(CC):
            t_ = atn_pool.tile([128, T], BF16, name=f"ATn_{cp}", tag=f"ATn{cp}")
            ATn.append(t_)

        est_pool = ctx.enter_context(tc.tile_pool(name="est", bufs=10))
        nrm_pool = ctx.enter_context(tc.tile_pool(name="nrm", bufs=4))
        y_pool = ctx.enter_context(tc.tile_pool(name="ysb", bufs=2))

        def attention(hp, ib, tail=False, filler=()):
            # filler: PE work (proj chains) injected between jc iterations so
            # the tensor engine never idles while ScalarE runs exp (idle PE
            # re-arms the HAM clock gate and everything drops to half speed)
            filler = list(filler)
            qt = qkT[hp]
            kt = qkT[6 + hp]
            po = {}
            for s in range(2):  # head 2*hp + s
                po[s] = ps_acc.tile([65, 512], F32, name=f"ps_ot_{hp}_{ib}_{s}", tag="ot", bufs=2)
            njc = 4 * (ib + 1)
            fill_at = (
                {(i * njc) // len(filler) for i in range(len(filler))}
                if filler
                else set()
            )
            for jc in range(njc):
                r = jc - 4 * ib
                col0 = max(r, 0) * 128
                # merged pair tile: head A in cols [0:512], head B in [512:1024]
                pst = ps_work.tile([128, 1024], F32, name=f"ps_st_{hp}_{ib}_{jc}", tag="ps")
                for s in range(2):
                    r0 = 64 * s
                    # row-packed pair: s=0 uses PE rows 0-63, s=1 rows 64-127
                    nc.tensor.matmul(
                        pst[:, 512 * s + col0 : 512 * s + 512],
                        kt[r0 : r0 + 64, jc * 128 : (jc + 1) * 128],
                        qt[r0 : r0 + 64, ib * 512 + col0 : (ib + 1) * 512],
                        start=True,
                        stop=True,
                    )
                est = est_pool.tile([128, 1024], BF16, name=f"est_{hp}_{ib}_{jc}", tag="est")
                nc.scalar.activation(
                    est.rearrange("p (a f) -> p a f", a=2)[:, :, col0:512],
                    pst.rearrange("p (a f) -> p a f", a=2)[:, :, col0:512],
                    AF.Exp,
                    scale=SCALE,
                )
                if r >= 0:
                    for s in range(2):
                        # mask the diagonal 128x128 sub-block (multiplicative)
                        nc.vector.tensor_tensor(
                            est[:, 512 * s + col0 : 512 * s + col0 + 128],
                            est[:, 512 * s + col0 : 512 * s + col0 + 128],
                            tri[:],
                            op=ALU.mult,
                        )
                if filler and jc in fill_at:
                    # emitted between the exp issue and this jc's PV pair so
                    # the chain runs on the PE while ScalarE computes the exp
                    filler.pop(0)()
                for s in range(2):
                    h = 2 * hp + s
                    nc.tensor.matmul(
                        po[s][:, col0:512],
                        Vp[jc][:, h * 65 : h * 65 + 65],
                        est[:, 512 * s + col0 : 512 * s + 512],
                        start=(jc == 0),
                        stop=(jc == njc - 1),
                    )
            for f in filler:
                f()
            # normalization: ATn rows = OT'[0:64] / Z  (Z = row 64).
            # Copy OT' to SBUF right away so the PSUM slot frees in ~0.7us;
            # the (long-latency, off-critical-path) normalization then runs
            # entirely from SBUF: gpsimd scatters Z across 128 partitions,
            # fast-approx reciprocal at [128,4], gather back, broadcast.
            if tail:
                # Last pair: nothing follows, so the po banks can stay held.
                # Skip the slow gpsimd scatter/gather/broadcast chain: copy
                # the Z rows to SBUF, broadcast them across 64 partitions with
                # an fp32 PE matmul (ones outer product; PE is idle here and
                # this also keeps the HAM clock gate open), reciprocal at
                # [64,512], and drain ATn straight from PSUM with a fused
                # scalar_tensor_tensor multiply.
                zrow = [
                    nrm_pool.tile([65, 512], F32, name=f"ztl_{s}", tag=f"ztl{s}", bufs=1)
                    for s in range(2)
                ]
                zbps = ps_work.tile([128, 1024], F32, name="zb_tail", tag="ps")
                for s in range(2):
                    # scalar engine is idle at the tail; DVE is not
                    nc.scalar.copy(zrow[s][64:65, :], po[s][64:65, :])
                    nc.tensor.matmul(
                        zbps[0:64, 512 * s : 512 * s + 512],
                        ones65[64:65, :],
                        zrow[s][64:65, :],
                        start=True,
                        stop=True,
                    )
                # dummy matmuls bridge the norm window so the HAM clock gate
                # stays open for the final proj chains
                for i in range(8):
                    wmt = ps_acc.tile([128, 128], F32, name=f"wmt_{i}", tag="acc")
                    nc.tensor.matmul(wmt[:], wseed[:], wseed[:], start=True, stop=True)
                for s in range(2):
                    zbr = nrm_pool.tile([64, 512], F32, name=f"zbr_{s}", tag=f"zbr{s}", bufs=1)
                    nc.vector.reciprocal_approx_fast(
                        out=zbr[:], in_=zbps[0:64, 512 * s : 512 * s + 512]
                    )
                    nc.vector.scalar_tensor_tensor(
                        ATn[hp][64 * s : 64 * s + 64, ib * 512 : (ib + 1) * 512],
                        po[s][0:64, :],
                        0.0,
                        zbr[:],
                        op0=ALU.bypass,
                        op1=ALU.mult,
                    )
                return
            otu = [
                nrm_pool.tile([65, 512], F32, name=f"otu_{hp}_{ib}_{s}", tag=f"otu{s}")
                for s in range(2)
            ]
            for s in range(2):
                nc.vector.tensor_copy(otu[s][:], po[s][:, :])
            # batched Z chain for both heads: one reciprocal pass
            # (zs2[p, s*4+c] = Z_s[4p+c], gather inverts the same mapping)
            zs2 = nrm_pool.tile([128, 8], F32, name=f"zs_{hp}_{ib}", tag="zs")
            for s in range(2):
                nc.gpsimd.dma_start(zs2[:, 4 * s : 4 * s + 4], otu[s][64:65, :])
            zr2 = nrm_pool.tile([128, 8], F32, name=f"zr_{hp}_{ib}", tag="zr")
            nc.vector.reciprocal_approx_fast(out=zr2[:], in_=zs2[:])
            zinv = [
                nrm_pool.tile([1, 512], F32, name=f"zinv_{hp}_{ib}_{s}", tag=f"zinv{s}")
                for s in range(2)
            ]
            for s in range(2):
                nc.gpsimd.dma_start(zinv[s][:], zr2[:, 4 * s : 4 * s + 4])
            for s in range(2):
                zb = nrm_pool.tile([64, 512], F32, name=f"zb_{hp}_{ib}_{s}", tag=f"zb{s}")
                nc.gpsimd.partition_broadcast(zb[:], zinv[s][:])
                nc.vector.tensor_tensor(
                    ATn[hp][64 * s : 64 * s + 64, ib * 512 : (ib + 1) * 512],
                    otu[s][0:64, :],
                    zb[:],
                    op=ALU.mult,
                )

        # proj is emitted as per-(tp, oc) chains used as PE filler inside the
        # exp-bound attention phase. tp 0-3: full 6-link chains; tp 4-7 split
        # in two 3-link sessions with an SBUF accumulator so session A can run
        # before the last head-pairs' ATn exist.
        yacc_pool = ctx.enter_context(tc.tile_pool(name="yacc", bufs=1))
        y_acc = {}
        y_sbs = {}

        def _proj_links(tp, oc, cps, start, stop):
            py = ps_acc.tile([128, 384], F32, name=f"ps_y_{tp}_{oc}", tag="acc")
            for i, cp in enumerate(cps):
                nc.tensor.matmul(
                    py[:],
                    ATn[cp][:, tp * 128 : (tp + 1) * 128],
                    Wp[cp][:, oc * 384 : (oc + 1) * 384],
                    start=start and (i == 0),
                    stop=stop and (i == len(cps) - 1),
                )
            return py

        def _maybe_dma(tp):
            if y_sbs[tp]["done"] == 2:
                nc.sync.dma_start(y_d[tp * 128 : (tp + 1) * 128, :], y_sbs[tp]["t"][:])

        def proj_full(tp, oc):
            def emit():
                py = _proj_links(tp, oc, list(range(CC)), True, True)
                if tp not in y_sbs:
                    y_sbs[tp] = {
                        "t": y_pool.tile([128, C], BF16, name=f"y_sb_{tp}", tag="y_sb"),
                        "done": 0,
                    }
                nc.vector.scalar_tensor_tensor(
                    y_sbs[tp]["t"][:, oc * 384 : (oc + 1) * 384],
                    py[:],
                    0.0,
                    bpb[:, oc * 384 : (oc + 1) * 384],
                    op0=ALU.bypass,
                    op1=ALU.add,
                )
                y_sbs[tp]["done"] += 1
                _maybe_dma(tp)
            return emit

        def proj_sessA(tp, oc):
            def emit():
                py = _proj_links(tp, oc, [0, 1, 2], True, True)
                if tp not in y_acc:
                    y_acc[tp] = yacc_pool.tile(
                        [128, C], F32, name=f"y_acc_{tp}", tag=f"ya{tp}"
                    )
                # y_acc = partial + output bias (bias folded here once)
                nc.vector.scalar_tensor_tensor(
                    y_acc[tp][:, oc * 384 : (oc + 1) * 384],
                    py[:],
                    0.0,
                    bpb[:, oc * 384 : (oc + 1) * 384],
                    op0=ALU.bypass,
                    op1=ALU.add,
                )
            return emit

        def proj_sessB(tp, oc):
            def emit():
                py = _proj_links(tp, oc, [3, 4, 5], True, True)
                if tp not in y_sbs:
                    y_sbs[tp] = {
                        "t": y_pool.tile([128, C], BF16, name=f"y_sb_{tp}", tag="y_sb"),
                        "done": 0,
                    }
                nc.vector.scalar_tensor_tensor(
                    y_sbs[tp]["t"][:, oc * 384 : (oc + 1) * 384],
                    py[:],
                    0.0,
                    y_acc[tp][:, oc * 384 : (oc + 1) * 384],
                    op0=ALU.bypass,
                    op1=ALU.add,
                )
                y_sbs[tp]["done"] += 1
                _maybe_dma(tp)
            return emit

        # Emission schedule. attention(hp, 0) ib=0 needs qkT pair hp tb0,
        # Vp[0..3] vc=0 for hp<3 (vc=1 for hp>=3); attention(hp, 1) needs the
        # full qkT pair + Vp[0..7]. All remaining chain work (later qk chains,
        # v halves, transposes tp4-7, proj) is spread as PE filler inside the
        # exp-bound attention jc loops so the tensor engine never starves
        # (which would also re-arm the HAM throttle). Every filler is emitted
        # only where its inputs are already resident.
        for tp in range(4):
            v_chunk(tp)
        qk_pair(0, tbs=(0,))
        qk_pair(1, tbs=(0,))
        for tp in range(4, 8):
            transpose_x(tp)
        attention(0, 0, filler=[v_half(4, 0), v_half(4, 1), v_half(5, 0), v_half(5, 1)])
        attention(1, 0, filler=[qk_chain(2, 0, 0), qk_chain(2, 0, 1), v_half(6, 0), v_half(6, 1)])
        attention(2, 0, filler=[qk_chain(3, 0, 0), qk_chain(3, 0, 1), v_half(7, 0), v_half(7, 1)])
        attention(3, 0, filler=[qk_chain(4, 0, 0), qk_chain(4, 0, 1), qk_chain(0, 1, 0), qk_chain(0, 1, 1)])
        attention(4, 0, filler=[qk_chain(5, 0, 0), qk_chain(5, 0, 1), qk_chain(1, 1, 0), qk_chain(1, 1, 1)])
        attention(5, 0, filler=[qk_chain(2, 1, 0), qk_chain(2, 1, 1), qk_chain(3, 1, 0), qk_chain(3, 1, 1)])
        attention(0, 1, filler=[qk_chain(4, 1, 0), qk_chain(4, 1, 1), proj_full(0, 0), proj_full(0, 1)])
        attention(1, 1, filler=[qk_chain(5, 1, 0), qk_chain(5, 1, 1), proj_full(1, 0), proj_full(1, 1)])
        phase_ctx.close()  # release xt/xsb/w SBUF (all readers emitted)
        attention(2, 1, filler=[proj_full(2, 0), proj_full(2, 1)])
        attention(3, 1, filler=[proj_full(3, 0), proj_full(3, 1)])
        attention(
            4, 1,
            filler=[proj_sessA(4, 0), proj_sessA(4, 1), proj_sessA(5, 0), proj_sessA(5, 1)],
        )
        attention(
            5, 1, tail=True,
            filler=[proj_sessA(6, 0), proj_sessA(6, 1), proj_sessA(7, 0), proj_sessA(7, 1)],
        )
        for tp in range(4, 8):
            proj_sessB(tp, 0)()
            proj_sessB(tp, 1)()


def kernel(x, W_attn, b_attn, W_proj, b_proj, _trace=False, _trace_kwargs=None):
    import ml_dtypes

    bf16 = ml_dtypes.bfloat16
    x = np.asarray(x)
    W_attn = np.asarray(W_attn)
    b_attn = np.asarray(b_attn)
    W_proj = np.asarray(W_proj)
    b_proj = np.asarray(b_proj)
    # v-bias composes linearly through the projection (softmax rows sum to 1):
    # y = (softmax @ (xWv)) W_proj + (b_v W_proj + b_proj)
    bp_eff = (
        b_attn[2 * C :].astype(np.float64) @ W_proj.astype(np.float64)
        + b_proj.astype(np.float64)
    ).astype(np.float32)

    xb = np.ascontiguousarray(x.astype(bf16))
    W_attnb = np.ascontiguousarray(W_attn.astype(bf16))
    ba_qk = np.ascontiguousarray(b_attn[: 2 * C].astype(bf16)).reshape(1, 2 * C)
    W_projb = np.ascontiguousarray(W_proj.astype(bf16))
    bp_eff = np.ascontiguousarray(bp_eff).reshape(1, C)

    if "prog" not in _PROGRAM_CACHE:
        _PROGRAM_CACHE["prog"] = build_program()
    nc = _PROGRAM_CACHE["prog"]

    in_maps = [
        {
            "x": np.ascontiguousarray(xb[b]),
            "W_attn": W_attnb,
            "b_attn": ba_qk,
            "W_proj": W_projb,
            "b_proj": bp_eff,
        }
        for b in range(NCORES)
    ]
    res = run_bass_kernel_spmd(
        nc,
        in_maps,
        core_ids=list(range(NCORES)),
        trace=_trace,
        **(_trace_kwargs or {}),
    )
    out = np.stack(
        [res.results[b]["y"].astype(np.float32) for b in range(NCORES)], axis=0
    )
    if _trace:
        return out, res
    return out


if __name__ == "__main__":
    rng = np.random.default_rng(0)
    x = rng.standard_normal((NCORES, T, C)).astype(np.float32)
    W_attn = (rng.standard_normal((C, 3 * C)) * 0.02).astype(np.float32)
    b_attn = np.zeros(3 * C, np.float32)
    W_proj = (rng.standard_normal((C, C)) * 0.02).astype(np.float32)
    b_proj = np.zeros(C, np.float32)
    y = kernel(x=x, W_attn=W_attn, b_attn=b_attn, W_proj=W_proj, b_proj=b_proj)
    print("out", y.shape, y.dtype, np.abs(y).max())
